# revision 39
# baseline (speedup 1.0000x reference)
"""LocallyConnected2d Trainium2 kernel (8-core SPMD).

out[b,o,p,q] = sum_{i,kh,kw} x[b, i, 2p+kh, 2q+kw] * weight[0, o, i, p, q, kh*3+kw]

Shipped variant "v14P" (see _build_nc_v14). Key facts learned on this HW
(axon-tunneled TRN2, no NTFF trace available; all numbers from For_i
repeat-loop slope timing):
- Shard the H' (=31) output-row dim across 8 cores (4 rows/core; core 7
  gets one duplicated padding row). This splits the dominant traffic —
  the 35.4MB per-location weight — 8 ways.
- For_i iterations are barrier-separated: NOTHING overlaps across
  iterations, so per-iter time = the single-iteration critical path.
- A dma_start costs ~18ns per partition-row of descriptor generation,
  serialized per HWDGE ring ([128, C] DMA ~2.3us; [32, C] ~0.6us), and
  the two HWDGE rings (sync, scalar) generate descriptors in parallel.
  Streaming is ~330 GB/s aggregate (shared across rings).
- Matmul cost ~ N(moving cols) x pe_cycle (PE at ~1.2GHz mid-p-state for
  these short bursts) with K=128 back-to-back matmuls pipelining their
  weight loads; K=32 matmuls do NOT pipeline (~+350ns each), so the
  32-row contraction remainder is handled with HALF as many K=32
  matmuls via the paired-quadrant trick (see v14 comment).
- scalar.copy (ACT engine) drags a ~1.4us InstLoadActFuncSet into every
  loop iteration -> PSUM drains are DVE-only.
- fp16 everywhere (tolerance 2e-2; measured rel err ~3.4e-4). fp16
  output halves out bytes vs fp32.
- Layout: contraction 288 = 128 + 128 + 32 so the bulk input rides all
  128 partitions / 16 SBUF ports (v10's 96-partition tiles idle 4 of 16
  and stream ~25% slower); quad-major column order with the 4 quarter
  DMAs alternating sync/scalar rings so compute on quarter q overlaps
  the stream of quarter q+1.
Variant history: v2 exact fp32 ~39us; v10 (96-part, fp32 out) ~20.7us;
v12 (single DMA, serial) ~32us; v13C ~21-25us; v14P ~19.4-20.8us under
the same conditions as the v10 measurement (ambient HBM contention on
this box drifts +/-3us between rounds; the harness baseline for v10 was
24.3us). Also tried and rejected (same-round A/B, all within noise or
worse): v14S per-block c2 (+3us), v14F one-copy-per-pair full-psum
drain (the +524KB out bytes beat the ~2us DVE saving), v14Pq
quarter-wise out pieces (wash), 2-piece input halves (+1-3us), c2 DMA
on the gpsimd/SWDGE ring (+1-2us: SWDGE fixed cost + out-ring
interference), DVE/ACT copy split "v14Pa" (wash: the per-iteration
InstLoadActFuncSet — which re-emits inside the For_i body even after a
pre-loop warm copy, verified in BIR, and which the Tile scheduler
hoists AHEAD of the scalar-ring dma_start issues, delaying Q1/Q3 —
cancels the ~3us DVE relief). A third HWDGE ring does not exist:
vector/tensor.dma_start raise (hwdge_engines = {SP, Activation}).
"v15" (pair-major cols, uneven 3+2+2+1-pair DMA pieces so the final
piece gates only ONE pair of tail compute) also measured as a wash in
round 1 and clearly last in a second independent round — the last
piece still queues behind two descriptor generations on its ring, so
the tail saving never reaches the critical path. A two-round
tiebreaker of the three co-leaders left v14P and v14Pa statistically
identical (each wins ~half the min/p25/med stats both rounds); v14P
is shipped as the safer of the two (v14Pa's per-iteration ACT-table
load schedules ahead of the scalar-ring dma_start issues, a latent
cost under heavier ambient contention). A 3-piece input split
("v15t", pairs 3+3+2, one fewer scalar-ring descriptor gen) also
split the stats vs v14P (won min by 1us, lost med by 3us) — the
2/3/4/5-piece corners are now ALL sampled and land within the +/-1.5us
ambient noise of each other; piece-count is saturated as a knob.
"""

import os
import numpy as np
import ml_dtypes

import concourse.bacc as bacc
import concourse.mybir as mybir
import concourse.tile as tile
from concourse.bass_utils import run_bass_kernel_spmd

# Problem shapes (hardcoded per contract).
B, CI, H, W = 8, 32, 64, 64
CO = 32
KH = KW = 3
DH = DW = 2
HO = WO = 31
N_CORES = 8
RPC = 4                 # padded H'-rows per core
L = RPC * WO            # 124 locations per core
IK = CI * KH * KW       # 288 contraction
NCHUNK = 3
CK = IK // NCHUNK       # 96 partitions per chunk
GROUPS = RPC            # one compute/DMA group per H'-row
GL = L // GROUPS        # 31 locations per group

W_COLS = L * NCHUNK * CO     # 11904
WIN_COLS = L * NCHUNK * B    # 2976
OUT_COLS = L * B             # 992

_ROWS_PADDED = [[min(4 * c + j, HO - 1) for j in range(RPC)] for c in range(N_CORES)]

_NC_CACHE = {}


V2_GOUT = 256               # psum cols per group in v2: 8 col-blocks x 32 (o)
V2_OUT_COLS = V2_GOUT * GROUPS

# v4: blocked matmuls — BLK locations share one matmul (out is a BLK x BLK
# grid of [b, o] tiles; only the diagonal is useful, extracted host-side).
# fp32r needs moving free dim >= 256 for the 1 cycle/row fast path.
GLP = 32                    # padded locs per group (31 real + 1 dup)
V4_CFG = {
    "v4r": (mybir.dt.float32r, 8, np.float32),
    "v4b": (mybir.dt.bfloat16, 4, ml_dtypes.bfloat16),
    "v4b8": (mybir.dt.bfloat16, 8, ml_dtypes.bfloat16),
}


def _build_nc_v4(repeat, variant):
    dt, BLK, _ = V4_CFG[variant]
    NBLK = GLP // BLK
    gw = NCHUNK * GLP * CO   # 3072 weight cols per group
    gwin = NCHUNK * GLP * B  # 768 win cols per group
    bout = BLK * CO          # out cols per block
    orows = B * BLK          # out rows per block
    out_cols = GROUPS * NBLK * bout

    nc = bacc.Bacc("TRN2", target_bir_lowering=False)
    wT = nc.dram_tensor("wT", [GROUPS * CK, gw], dt, kind="ExternalInput")
    winT = nc.dram_tensor("winT", [GROUPS * CK, gwin], dt, kind="ExternalInput")
    out = nc.dram_tensor("out", [orows, out_cols], mybir.dt.float32, kind="ExternalOutput")

    with tile.TileContext(nc) as tc:
        with (
            tc.tile_pool(name="wp", bufs=3) as wp,
            tc.tile_pool(name="winp", bufs=3) as winp,
            tc.tile_pool(name="pp", bufs=4, space="PSUM") as pp,
            tc.tile_pool(name="op", bufs=4) as op,
        ):
            def body():
                for g in range(GROUPS):
                    wt = wp.tile([CK, gw], dt, tag="wt", name="wt")
                    nc.sync.dma_start(wt[:], wT.ap()[g * CK:(g + 1) * CK, :])
                    wint = winp.tile([CK, gwin], dt, tag="wint", name="wint")
                    nc.sync.dma_start(wint[:], winT.ap()[g * CK:(g + 1) * CK, :])

                    for bl in range(NBLK):
                        ps = pp.tile([orows, bout], mybir.dt.float32, tag="ps", name="ps")
                        for c in range(NCHUNK):
                            nc.tensor.matmul(
                                ps[:],
                                lhsT=wint[:, c * (GLP * B) + bl * (BLK * B):
                                          c * (GLP * B) + (bl + 1) * (BLK * B)],
                                rhs=wt[:, c * (GLP * CO) + bl * bout:
                                       c * (GLP * CO) + (bl + 1) * bout],
                                start=(c == 0),
                                stop=(c == NCHUNK - 1),
                            )
                        ot = op.tile([orows, bout], mybir.dt.float32, tag="ot", name="ot")
                        nc.vector.tensor_copy(ot[:], ps[:])
                        nc.sync.dma_start(
                            out.ap()[:, (g * NBLK + bl) * bout:(g * NBLK + bl + 1) * bout],
                            ot[:],
                        )

            if repeat == 1:
                body()
            else:
                with tc.For_i(0, repeat, 1):
                    body()
    nc.compile()
    return nc


def _build_nc_v5(repeat=1):
    """fp32 exact; all DMAs 128-partition; contraction 128+128+32 with the
    32-row remainder of all 4 groups packed into one 128-row tile."""
    gw = GL * CO     # 992 weight cols per (group, chunk)
    gwin = GL * B    # 248 win cols per (group, chunk)
    nc = bacc.Bacc("TRN2", target_bir_lowering=False)
    w01 = nc.dram_tensor("w01", [GROUPS * 2 * 128, gw], mybir.dt.float32, kind="ExternalInput")
    win01 = nc.dram_tensor("win01", [GROUPS * 2 * 128, gwin], mybir.dt.float32, kind="ExternalInput")
    w2 = nc.dram_tensor("w2", [GROUPS * 32, gw], mybir.dt.float32, kind="ExternalInput")
    win2 = nc.dram_tensor("win2", [GROUPS * 32, gwin], mybir.dt.float32, kind="ExternalInput")
    out = nc.dram_tensor("out", [GROUPS * 128, V2_GOUT], mybir.dt.float32, kind="ExternalOutput")

    with tile.TileContext(nc) as tc:
        with (
            tc.tile_pool(name="wp", bufs=3) as wp,
            tc.tile_pool(name="winp", bufs=3) as winp,
            tc.tile_pool(name="pp", bufs=2, space="PSUM") as pp,
            tc.tile_pool(name="op", bufs=2) as op,
        ):
            def body():
                for g in range(GROUPS):
                    wts, wints = [], []
                    for cc in range(2):
                        wt = wp.tile([128, gw], mybir.dt.float32, tag=f"wt{cc}", name=f"wt{cc}")
                        nc.sync.dma_start(
                            wt[:], w01.ap()[(g * 2 + cc) * 128:(g * 2 + cc + 1) * 128, :])
                        wint = winp.tile([128, gwin], mybir.dt.float32, tag=f"wint{cc}", name=f"wint{cc}")
                        nc.sync.dma_start(
                            wint[:], win01.ap()[(g * 2 + cc) * 128:(g * 2 + cc + 1) * 128, :])
                        wts.append(wt)
                        wints.append(wint)
                    w2t = wp.tile([32, gw], mybir.dt.float32, tag="w2t", name="w2t")
                    nc.sync.dma_start(w2t[:], w2.ap()[g * 32:(g + 1) * 32, :])
                    win2t = winp.tile([32, gwin], mybir.dt.float32, tag="win2t", name="win2t")
                    nc.sync.dma_start(win2t[:], win2.ap()[g * 32:(g + 1) * 32, :])

                    pss = [
                        pp.tile([128, V2_GOUT], mybir.dt.float32,
                                tag=f"ps{j}", name=f"ps{j}", bufs=2)
                        for j in range(4)
                    ]
                    for l in range(GL):
                        j = l % 4
                        blk = l // 4
                        dst = pss[j][32 * j:32 * j + B, blk * CO:(blk + 1) * CO]
                        for cc in range(2):
                            nc.tensor.matmul(
                                dst,
                                lhsT=wints[cc][:, l * B:(l + 1) * B],
                                rhs=wts[cc][:, l * CO:(l + 1) * CO],
                                start=(cc == 0),
                                stop=False,
                                tile_position=(0, 32 * j),
                            )
                        nc.tensor.matmul(
                            dst,
                            lhsT=win2t[:, l * B:(l + 1) * B],
                            rhs=w2t[:, l * CO:(l + 1) * CO],
                            start=False,
                            stop=True,
                            tile_position=(0, 32 * j),
                        )

                    ot = op.tile([128, V2_GOUT], mybir.dt.float32, tag="ot", name="ot")
                    for j in range(4):
                        nc.vector.tensor_copy(
                            ot[32 * j:32 * (j + 1), :],
                            pss[j][32 * j:32 * (j + 1), :],
                        )
                    nc.sync.dma_start(out.ap()[g * 128:(g + 1) * 128, :], ot[:])

            if repeat == 1:
                body()
            else:
                with tc.For_i(0, repeat, 1):
                    body()
    nc.compile()
    return nc


def _host_prep_v5(x, weight):
    x = np.ascontiguousarray(np.asarray(x, dtype=np.float32))
    weight = np.ascontiguousarray(np.asarray(weight, dtype=np.float32))
    wins = np.stack(
        [x[:, :, kh:kh + DH * HO:DH, kw:kw + DW * WO:DW]
         for kh in range(KH) for kw in range(KW)],
        axis=-1,
    )
    W2 = weight[0].transpose(1, 4, 2, 3, 0).reshape(IK, HO, WO, CO)
    W3 = wins.transpose(1, 4, 2, 3, 0).reshape(IK, HO, WO, B)
    in_maps = []
    for c in range(N_CORES):
        rows = _ROWS_PADDED[c]
        wsel = W2[:, rows]       # (288, 4, 31, CO)
        winsel = W3[:, rows]     # (288, 4, 31, B)
        # w01 rows: (g, c01, 128) ; cols (l, o)
        w01 = wsel[:256].reshape(2, 128, GROUPS, GL * CO).transpose(2, 0, 1, 3)
        win01 = winsel[:256].reshape(2, 128, GROUPS, GL * B).transpose(2, 0, 1, 3)
        w2 = wsel[256:].reshape(32, GROUPS, GL * CO).transpose(1, 0, 2)
        win2 = winsel[256:].reshape(32, GROUPS, GL * B).transpose(1, 0, 2)
        in_maps.append({
            "w01": np.ascontiguousarray(w01.reshape(GROUPS * 2 * 128, GL * CO)),
            "win01": np.ascontiguousarray(win01.reshape(GROUPS * 2 * 128, GL * B)),
            "w2": np.ascontiguousarray(w2.reshape(GROUPS * 32, GL * CO)),
            "win2": np.ascontiguousarray(win2.reshape(GROUPS * 32, GL * B)),
        })
    return in_maps


def _assemble_v5(results):
    out = np.empty((B, CO, HO, WO), np.float32)
    qs = np.arange(WO)
    for c in range(N_CORES):
        nreal = RPC if c < N_CORES - 1 else HO - 4 * (N_CORES - 1)
        buf = np.asarray(results[c]["out"])      # [GROUPS*128, 256]
        b5 = buf.reshape(GROUPS, 4, 32, 8, CO)   # (g, strip, 32row, blk, o)
        res = b5[:, qs % 4, :B, qs // 4, :]      # (g?, ...) advanced idx
        # advanced indices qs%4 (dim1) and qs//4 (dim3) -> (31, GROUPS, B, CO)
        out[:, :, 4 * c:4 * c + nreal, :] = res.transpose(2, 3, 1, 0)[:, :, :nreal, :]
    return out


V89_BLK = 4
V89_NBLK = GLP // V89_BLK            # 8 blocks of 4 locs per group
V89_GW = NCHUNK * GLP * CO           # 3072 weight cols per group
V89_GWIN = NCHUNK * GLP * B          # 768 win cols per group
V89_BOUT = V89_BLK * CO              # 128 out cols per block
V89_OROWS = B * V89_BLK              # 32 out rows
V89_OUTC = GROUPS * V89_NBLK * V89_BOUT  # 4096


def _build_nc_v89(repeat=1, three_term=False, dt=None):
    """16-bit blocked kernel, minimal DMA count, split across both HWDGE
    rings. three_term=True computes w≈wh+wl, win≈vh+vl and accumulates
    vh·wh + vh·wl + vl·wh (16-bit products are exact in fp32 -> ~1e-5 rel err).
    """
    if dt is None:
        dt = mybir.dt.bfloat16
    W = GROUPS * V89_GW
    WIN = GROUPS * V89_GWIN
    nc = bacc.Bacc("TRN2", target_bir_lowering=False)
    wh_d = nc.dram_tensor("wh", [CK, W], dt, kind="ExternalInput")
    winh_d = nc.dram_tensor("winh", [CK, WIN], dt, kind="ExternalInput")
    if three_term:
        wl_d = nc.dram_tensor("wl", [CK, W], dt, kind="ExternalInput")
        winl_d = nc.dram_tensor("winl", [CK, WIN], dt, kind="ExternalInput")
    out = nc.dram_tensor("out", [V89_OROWS, V89_OUTC], mybir.dt.float32, kind="ExternalOutput")

    half = W // 2  # 2 groups per ring half
    with tile.TileContext(nc) as tc:
        with (
            tc.tile_pool(name="wp", bufs=2) as wp,
            tc.tile_pool(name="winp", bufs=2) as winp,
            tc.tile_pool(name="pp", bufs=4, space="PSUM") as pp,
            tc.tile_pool(name="op", bufs=2) as op,
        ):
            def body():
                # weight: groups 0-1 via SP ring, groups 2-3 via ACT ring,
                # one piece per group -> compute starts after 1/4 of bytes
                wh = wp.tile([CK, W], dt, tag="wh", name="wh")
                for g in range(2):
                    nc.sync.dma_start(
                        wh[:, g * V89_GW:(g + 1) * V89_GW],
                        wh_d.ap()[:, g * V89_GW:(g + 1) * V89_GW])
                for g in range(2, 4):
                    nc.scalar.dma_start(
                        wh[:, g * V89_GW:(g + 1) * V89_GW],
                        wh_d.ap()[:, g * V89_GW:(g + 1) * V89_GW])
                winh = winp.tile([CK, WIN], dt, tag="winh", name="winh")
                nc.sync.dma_start(winh[:, :WIN // 2], winh_d.ap()[:, :WIN // 2])
                nc.scalar.dma_start(winh[:, WIN // 2:], winh_d.ap()[:, WIN // 2:])
                if three_term:
                    wl = wp.tile([CK, W], dt, tag="wl", name="wl")
                    for g in range(2):
                        nc.scalar.dma_start(
                            wl[:, g * V89_GW:(g + 1) * V89_GW],
                            wl_d.ap()[:, g * V89_GW:(g + 1) * V89_GW])
                    for g in range(2, 4):
                        nc.sync.dma_start(
                            wl[:, g * V89_GW:(g + 1) * V89_GW],
                            wl_d.ap()[:, g * V89_GW:(g + 1) * V89_GW])
                    winl = winp.tile([CK, WIN], dt, tag="winl", name="winl")
                    nc.scalar.dma_start(winl[:, :WIN // 2], winl_d.ap()[:, :WIN // 2])
                    nc.sync.dma_start(winl[:, WIN // 2:], winl_d.ap()[:, WIN // 2:])

                ot = op.tile([V89_OROWS, V89_OUTC], mybir.dt.float32, tag="ot", name="ot")
                for g in range(GROUPS):
                    for bl in range(V89_NBLK):
                        ps = pp.tile([V89_OROWS, V89_BOUT], mybir.dt.float32, tag="ps", name="ps")
                        first = True
                        for c in range(NCHUNK):
                            lo = g * V89_GWIN + c * (GLP * B) + bl * (V89_BLK * B)
                            ro = g * V89_GW + c * (GLP * CO) + bl * V89_BOUT
                            lhs_h = winh[:, lo:lo + V89_BLK * B]
                            rhs_h = wh[:, ro:ro + V89_BOUT]
                            terms = [(lhs_h, rhs_h)]
                            if three_term:
                                terms.append((lhs_h, wl[:, ro:ro + V89_BOUT]))
                                terms.append((winl[:, lo:lo + V89_BLK * B], rhs_h))
                            for ti, (lh, rh) in enumerate(terms):
                                last = (c == NCHUNK - 1) and (ti == len(terms) - 1)
                                nc.tensor.matmul(
                                    ps[:], lhsT=lh, rhs=rh,
                                    start=first, stop=last)
                                first = False
                        nc.vector.tensor_copy(
                            ot[:, (g * V89_NBLK + bl) * V89_BOUT:(g * V89_NBLK + bl + 1) * V89_BOUT],
                            ps[:])
                nc.gpsimd.dma_start(out.ap()[:, :], ot[:])

            if repeat == 1:
                body()
            else:
                with tc.For_i(0, repeat, 1):
                    body()
    nc.compile()
    return nc


def _host_prep_v89(x, weight, three_term=False, npdt=None):
    if npdt is None:
        npdt = ml_dtypes.bfloat16
    x = np.ascontiguousarray(np.asarray(x, dtype=np.float32))
    weight = np.ascontiguousarray(np.asarray(weight, dtype=np.float32))
    wins = np.stack(
        [x[:, :, kh:kh + DH * HO:DH, kw:kw + DW * WO:DW]
         for kh in range(KH) for kw in range(KW)],
        axis=-1,
    )
    W2 = weight[0].transpose(1, 4, 2, 3, 0).reshape(IK, HO, WO, CO)
    W3 = wins.transpose(1, 4, 2, 3, 0).reshape(IK, HO, WO, B)
    qpad = list(range(WO)) + [WO - 1]
    in_maps = []
    for c in range(N_CORES):
        rows = _ROWS_PADDED[c]
        wsel = W2[:, rows][:, :, qpad, :]       # (288, 4, 32, CO)
        winsel = W3[:, rows][:, :, qpad, :]     # (288, 4, 32, B)
        # -> [CK, (group, chunk, locp, {o|b})]
        wstk = np.stack([wsel[CK * cc:CK * (cc + 1)] for cc in range(NCHUNK)], axis=2)
        winstk = np.stack([winsel[CK * cc:CK * (cc + 1)] for cc in range(NCHUNK)], axis=2)
        # (CK, 4, chunk, 32, X) -> (CK, group*chunk*locp*X)
        wfull = wstk.reshape(CK, GROUPS * NCHUNK * GLP * CO)
        winfull = winstk.reshape(CK, GROUPS * NCHUNK * GLP * B)
        m = {}
        wh = wfull.astype(npdt)
        vh = winfull.astype(npdt)
        m["wh"] = np.ascontiguousarray(wh)
        m["winh"] = np.ascontiguousarray(vh)
        if three_term:
            m["wl"] = np.ascontiguousarray(
                (wfull - wh.astype(np.float32)).astype(npdt))
            m["winl"] = np.ascontiguousarray(
                (winfull - vh.astype(np.float32)).astype(npdt))
        in_maps.append(m)
    return in_maps


def _assemble_v89(results):
    BLK = V89_BLK
    NBLK = V89_NBLK
    out = np.empty((B, CO, HO, WO), np.float32)
    idx = np.arange(BLK)
    for c in range(N_CORES):
        nreal = RPC if c < N_CORES - 1 else HO - 4 * (N_CORES - 1)
        buf = np.asarray(results[c]["out"])          # [32, 4096]
        b6 = buf.reshape(BLK, B, GROUPS, NBLK, BLK, CO)
        d = b6[idx, :, :, :, idx, :]                 # (BLK, B, G, NBLK, CO)
        dd = d.transpose(1, 4, 2, 3, 0).reshape(B, CO, GROUPS, NBLK * BLK)
        out[:, :, 4 * c:4 * c + nreal, :] = dd[:, :, :nreal, :WO]
    return out


# ---------------------------------------------------------------------------
# v12: 128-partition layout. Contraction 288 = 128 + 128 + 32; c0/c1 live in
# one [128, 10240] tensor so the bulk DMA rides all 128 partitions / 16 SBUF
# ports (the 96-partition v10 layout idles 4 of 16). The 32-row remainder c2
# is a separate [32, 5120] tensor DMAed on the scalar HWDGE ring, concurrent
# with the sync-ring bulk DMA (matmul base-partition must be 0/32/64, so a
# 4-way partition fold of c2 is not expressible).
# Output is fp16 (tolerance is 2e-2; fp16 rounding ~5e-4) halving out bytes.
# Input is ~31KB/partition -> bufs=2 double-buffering fits, so iteration
# i+1's input DMA overlaps iteration i's compute in the steady state.
V12_LOCS = 128              # 4 padded p-rows x 32 padded q
V12_NBLK = 32               # blocks of BLK=4 locs
V12_BLK = 4
V12_W0 = 0                  # [128, 4096] c0 weight, cols (bl, j, o)
V12_V0 = 4096               # [128, 1024] c0 windows, cols (bl, j, b)
V12_W1 = 5120               # [128, 4096] c1 weight
V12_V1 = 9216               # [128, 1024] c1 windows
V12_COLS = 10240
V12_C2COLS = V12_NBLK * 160  # per block: 128 weight cols | 32 window cols
V12_OROWS = V12_BLK * B     # 32
V12_OUTC = V12_NBLK * V12_BLK * CO  # 4096


def _build_nc_v12(repeat=1, n_in_dma=1, c2_ring="scalar", copy_eng="vs",
                  skip_c2=False, skip_out=False):
    """c2_ring: scalar | sync_after | sync_before; copy_eng: vs|v|vg;
    skip_c2/skip_out: timing-only ablations (results wrong)."""
    dt = mybir.dt.float16
    nc = bacc.Bacc("TRN2", target_bir_lowering=False)
    wx = nc.dram_tensor("wx", [128, V12_COLS], dt, kind="ExternalInput")
    c2 = nc.dram_tensor("c2", [32, V12_C2COLS], dt, kind="ExternalInput")
    out = nc.dram_tensor("out", [V12_OROWS, V12_OUTC], dt, kind="ExternalOutput")
    with tile.TileContext(nc) as tc:
        with (
            tc.tile_pool(name="wp", bufs=2) as wp,
            tc.tile_pool(name="cp", bufs=2) as cp,
            tc.tile_pool(name="pp", bufs=4, space="PSUM") as pp,
            tc.tile_pool(name="op", bufs=2) as op,
        ):
            def load_c2(eng):
                t2 = cp.tile([32, V12_C2COLS], dt, tag="t2", name="t2")
                eng.dma_start(t2[:], c2.ap()[:, :])
                return t2

            def body():
                if c2_ring == "sync_before":
                    t2 = load_c2(nc.sync)
                t = wp.tile([128, V12_COLS], dt, tag="t", name="t")
                if n_in_dma == 1:
                    nc.sync.dma_start(t[:], wx.ap()[:, :])
                else:
                    step = V12_COLS // n_in_dma
                    for d in range(n_in_dma):
                        nc.sync.dma_start(
                            t[:, d * step:(d + 1) * step],
                            wx.ap()[:, d * step:(d + 1) * step])
                if c2_ring == "scalar":
                    t2 = load_c2(nc.scalar)
                elif c2_ring == "sync_after":
                    t2 = load_c2(nc.sync)
                ot = op.tile([V12_OROWS, V12_OUTC], dt, tag="ot", name="ot")
                for bl in range(V12_NBLK):
                    ps = pp.tile([V12_OROWS, V12_BLK * CO], mybir.dt.float32,
                                 tag="ps", name="ps")
                    nc.tensor.matmul(
                        ps[:],
                        lhsT=t[:, V12_V0 + 32 * bl:V12_V0 + 32 * bl + 32],
                        rhs=t[:, V12_W0 + 128 * bl:V12_W0 + 128 * bl + 128],
                        start=True, stop=skip_c2 and False)
                    nc.tensor.matmul(
                        ps[:],
                        lhsT=t[:, V12_V1 + 32 * bl:V12_V1 + 32 * bl + 32],
                        rhs=t[:, V12_W1 + 128 * bl:V12_W1 + 128 * bl + 128],
                        start=False, stop=bool(skip_c2))
                    if not skip_c2:
                        nc.tensor.matmul(
                            ps[:],
                            lhsT=t2[:, 160 * bl + 128:160 * bl + 160],
                            rhs=t2[:, 160 * bl:160 * bl + 128],
                            start=False, stop=True)
                    dst = ot[:, 128 * bl:128 * (bl + 1)]
                    if copy_eng == "v":
                        nc.vector.tensor_copy(dst, ps[:])
                    elif copy_eng == "vs":
                        (nc.vector.tensor_copy if bl % 2 == 0 else nc.scalar.copy)(dst, ps[:])
                    else:
                        (nc.vector.tensor_copy if bl % 2 == 0 else nc.gpsimd.tensor_copy)(dst, ps[:])
                    if not skip_out and bl == V12_NBLK - 8:
                        nc.gpsimd.dma_start(out.ap()[:, :3 * 1024], ot[:, :3 * 1024])
                if not skip_out:
                    nc.gpsimd.dma_start(out.ap()[:, 3 * 1024:], ot[:, 3 * 1024:])
                else:
                    nc.gpsimd.dma_start(out.ap()[:, :128], ot[:, :128])
            if repeat == 1:
                body()
            else:
                with tc.For_i(0, repeat, 1):
                    body()
    nc.compile()
    return nc


def _build_dma_probe(repeat, nparts, cols, ndma, rings=("sync",)):
    """Timing probe: input DMA only (+1 tiny out DMA), no compute.
    The cols are split into ndma pieces, round-robined over `rings`."""
    dt = mybir.dt.float16
    nc = bacc.Bacc("TRN2", target_bir_lowering=False)
    wx = nc.dram_tensor("wx", [nparts, cols], dt, kind="ExternalInput")
    out = nc.dram_tensor("out", [32, 128], dt, kind="ExternalOutput")
    with tile.TileContext(nc) as tc:
        with (
            tc.tile_pool(name="wp", bufs=2) as wp,
            tc.tile_pool(name="op", bufs=2) as op,
        ):
            def body():
                t = wp.tile([nparts, cols], dt, tag="t", name="t")
                step = cols // ndma
                for d in range(ndma):
                    eng = getattr(nc, rings[d % len(rings)])
                    eng.dma_start(t[:, d * step:(d + 1) * step],
                                  wx.ap()[:, d * step:(d + 1) * step])
                ot = op.tile([32, 128], dt, tag="ot", name="ot")
                nc.vector.tensor_copy(ot[:], t[:32, :128])
                nc.gpsimd.dma_start(out.ap()[:, :], ot[:])
            if repeat == 1:
                body()
            else:
                with tc.For_i(0, repeat, 1):
                    body()
    nc.compile()
    return nc


def _build_micro_probe(repeat, mode, cols=10240):
    """mode: tinyloop | dmapure | chain | mm | mmnoc2 | mmcopy"""
    dt = mybir.dt.float16
    nc = bacc.Bacc("TRN2", target_bir_lowering=False)
    wx = nc.dram_tensor("wx", [128, V12_COLS], dt, kind="ExternalInput")
    c2 = nc.dram_tensor("c2", [32, V12_C2COLS], dt, kind="ExternalInput")
    out = nc.dram_tensor("out", [V12_OROWS, V12_OUTC], dt, kind="ExternalOutput")
    with tile.TileContext(nc) as tc:
        with (
            tc.tile_pool(name="wp", bufs=2) as wp,
            tc.tile_pool(name="cp", bufs=1) as cp,
            tc.tile_pool(name="pp", bufs=4, space="PSUM") as pp,
            tc.tile_pool(name="op", bufs=2) as op,
        ):
            # persistent copies of inputs, loaded once
            tp = wp.tile([128, V12_COLS], dt, tag="tp", name="tp")
            nc.sync.dma_start(tp[:], wx.ap()[:, :])
            t2 = cp.tile([32, V12_C2COLS], dt, tag="t2", name="t2")
            nc.scalar.dma_start(t2[:], c2.ap()[:, :])
            # out written once so the loop body need not
            ot0 = op.tile([V12_OROWS, V12_OUTC], dt, tag="ot", name="ot0")
            nc.vector.tensor_copy(ot0[:], tp[:32, :V12_OUTC])
            nc.gpsimd.dma_start(out.ap()[:, :], ot0[:])

            def body():
                if mode == "tinyloop":
                    t = wp.tile([128, 64], dt, tag="t", name="t")
                    nc.sync.dma_start(t[:], wx.ap()[:, :64])
                    return
                if mode == "tiny32":
                    t = wp.tile([32, 64], dt, tag="t", name="t")
                    nc.sync.dma_start(t[:], wx.ap()[:32, :64])
                    return
                if mode == "mmc2m":
                    # inline c2 but operands from the 128-part tile at base 0
                    for bl in range(V12_NBLK):
                        ps = pp.tile([V12_OROWS, V12_BLK * CO], mybir.dt.float32,
                                     tag="ps", name="ps")
                        nc.tensor.matmul(
                            ps[:],
                            lhsT=tp[:, V12_V0 + 32 * bl:V12_V0 + 32 * bl + 32],
                            rhs=tp[:, V12_W0 + 128 * bl:V12_W0 + 128 * bl + 128],
                            start=True, stop=False)
                        nc.tensor.matmul(
                            ps[:],
                            lhsT=tp[:, V12_V1 + 32 * bl:V12_V1 + 32 * bl + 32],
                            rhs=tp[:, V12_W1 + 128 * bl:V12_W1 + 128 * bl + 128],
                            start=False, stop=False)
                        nc.tensor.matmul(
                            ps[:],
                            lhsT=tp[:32, V12_V1 + 32 * bl:V12_V1 + 32 * bl + 32],
                            rhs=tp[:32, V12_W1 + 128 * bl:V12_W1 + 128 * bl + 128],
                            start=False, stop=True)
                    return
                if mode == "dmapure":
                    t = wp.tile([128, cols], dt, tag="t", name="t")
                    nc.sync.dma_start(t[:], wx.ap()[:, :cols])
                    return
                if mode == "chain":
                    ot = op.tile([V12_OROWS, V12_OUTC], dt, tag="ot", name="ot")
                    for bl in range(V12_NBLK):
                        dst = ot[:, 128 * bl:128 * (bl + 1)]
                        src = tp[:32, 128 * bl:128 * (bl + 1)]
                        (nc.vector.tensor_copy if bl % 2 == 0 else nc.scalar.copy)(dst, src)
                        if bl == V12_NBLK - 8:
                            nc.gpsimd.dma_start(out.ap()[:, :3 * 1024], ot[:, :3 * 1024])
                    nc.gpsimd.dma_start(out.ap()[:, 3 * 1024:], ot[:, 3 * 1024:])
                    return
                # mm / mmnoc2 / mmcopy / mmc2b (c2 batched per quad of 8)
                ot = op.tile([V12_OROWS, V12_OUTC], dt, tag="ot", name="ot") \
                    if mode == "mmcopy" else None
                if mode == "mmc2b":
                    for q in range(4):
                        pss = []
                        for bl in range(8 * q, 8 * q + 8):
                            ps = pp.tile([V12_OROWS, V12_BLK * CO], mybir.dt.float32,
                                         tag=f"ps{bl % 8}", name="ps", bufs=1)
                            pss.append(ps)
                            nc.tensor.matmul(
                                ps[:],
                                lhsT=tp[:, V12_V0 + 32 * bl:V12_V0 + 32 * bl + 32],
                                rhs=tp[:, V12_W0 + 128 * bl:V12_W0 + 128 * bl + 128],
                                start=True, stop=False)
                            nc.tensor.matmul(
                                ps[:],
                                lhsT=tp[:, V12_V1 + 32 * bl:V12_V1 + 32 * bl + 32],
                                rhs=tp[:, V12_W1 + 128 * bl:V12_W1 + 128 * bl + 128],
                                start=False, stop=False)
                        for i, bl in enumerate(range(8 * q, 8 * q + 8)):
                            nc.tensor.matmul(
                                pss[i][:],
                                lhsT=t2[:, 160 * bl + 128:160 * bl + 160],
                                rhs=t2[:, 160 * bl:160 * bl + 128],
                                start=False, stop=True)
                    return
                for bl in range(V12_NBLK):
                    ps = pp.tile([V12_OROWS, V12_BLK * CO], mybir.dt.float32,
                                 tag="ps", name="ps")
                    nc.tensor.matmul(
                        ps[:],
                        lhsT=tp[:, V12_V0 + 32 * bl:V12_V0 + 32 * bl + 32],
                        rhs=tp[:, V12_W0 + 128 * bl:V12_W0 + 128 * bl + 128],
                        start=True, stop=False)
                    nc.tensor.matmul(
                        ps[:],
                        lhsT=tp[:, V12_V1 + 32 * bl:V12_V1 + 32 * bl + 32],
                        rhs=tp[:, V12_W1 + 128 * bl:V12_W1 + 128 * bl + 128],
                        start=False, stop=(mode == "mmnoc2"))
                    if mode != "mmnoc2":
                        nc.tensor.matmul(
                            ps[:],
                            lhsT=t2[:, 160 * bl + 128:160 * bl + 160],
                            rhs=t2[:, 160 * bl:160 * bl + 128],
                            start=False, stop=True)
                    if mode == "mmcopy":
                        dst = ot[:, 128 * bl:128 * (bl + 1)]
                        (nc.vector.tensor_copy if bl % 2 == 0 else nc.scalar.copy)(dst, ps[:])
            if repeat == 1:
                body()
            else:
                with tc.For_i(0, repeat, 1):
                    body()
    nc.compile()
    return nc


def _build_compute_probe(repeat):
    """Timing probe: v12 compute+copies+out, input loaded once outside loop."""
    dt = mybir.dt.float16
    nc = bacc.Bacc("TRN2", target_bir_lowering=False)
    wx = nc.dram_tensor("wx", [128, V12_COLS], dt, kind="ExternalInput")
    c2 = nc.dram_tensor("c2", [32, V12_C2COLS], dt, kind="ExternalInput")
    out = nc.dram_tensor("out", [V12_OROWS, V12_OUTC], dt, kind="ExternalOutput")
    with tile.TileContext(nc) as tc:
        with (
            tc.tile_pool(name="wp", bufs=1) as wp,
            tc.tile_pool(name="cp", bufs=1) as cp,
            tc.tile_pool(name="pp", bufs=4, space="PSUM") as pp,
            tc.tile_pool(name="op", bufs=2) as op,
        ):
            t = wp.tile([128, V12_COLS], dt, tag="t", name="t")
            nc.sync.dma_start(t[:], wx.ap()[:, :])
            t2 = cp.tile([32, V12_C2COLS], dt, tag="t2", name="t2")
            nc.scalar.dma_start(t2[:], c2.ap()[:, :])

            def body():
                ot = op.tile([V12_OROWS, V12_OUTC], dt, tag="ot", name="ot")
                for bl in range(V12_NBLK):
                    ps = pp.tile([V12_OROWS, V12_BLK * CO], mybir.dt.float32,
                                 tag="ps", name="ps")
                    nc.tensor.matmul(
                        ps[:],
                        lhsT=t[:, V12_V0 + 32 * bl:V12_V0 + 32 * bl + 32],
                        rhs=t[:, V12_W0 + 128 * bl:V12_W0 + 128 * bl + 128],
                        start=True, stop=False)
                    nc.tensor.matmul(
                        ps[:],
                        lhsT=t[:, V12_V1 + 32 * bl:V12_V1 + 32 * bl + 32],
                        rhs=t[:, V12_W1 + 128 * bl:V12_W1 + 128 * bl + 128],
                        start=False, stop=False)
                    nc.tensor.matmul(
                        ps[:],
                        lhsT=t2[:, 160 * bl + 128:160 * bl + 160],
                        rhs=t2[:, 160 * bl:160 * bl + 128],
                        start=False, stop=True)
                    dst = ot[:, 128 * bl:128 * (bl + 1)]
                    (nc.vector.tensor_copy if bl % 2 == 0 else nc.scalar.copy)(dst, ps[:])
                    if bl == V12_NBLK - 8:
                        nc.gpsimd.dma_start(out.ap()[:, :3 * 1024], ot[:, :3 * 1024])
                nc.gpsimd.dma_start(out.ap()[:, 3 * 1024:], ot[:, 3 * 1024:])
            if repeat == 1:
                body()
            else:
                with tc.For_i(0, repeat, 1):
                    body()
    nc.compile()
    return nc


def _host_prep_v12(x, weight):
    x = np.ascontiguousarray(np.asarray(x, dtype=np.float32))
    weight = np.ascontiguousarray(np.asarray(weight, dtype=np.float32))
    wins = np.stack(
        [x[:, :, kh:kh + DH * HO:DH, kw:kw + DW * WO:DW]
         for kh in range(KH) for kw in range(KW)],
        axis=-1,
    )
    W2 = weight[0].transpose(1, 4, 2, 3, 0).reshape(IK, HO, WO, CO)
    W3 = wins.transpose(1, 4, 2, 3, 0).reshape(IK, HO, WO, B)
    qpad = list(range(WO)) + [WO - 1]
    in_maps = []
    for c in range(N_CORES):
        rows = _ROWS_PADDED[c]
        wsel = W2[:, rows][:, :, qpad, :].reshape(IK, V12_LOCS, CO).astype(np.float16)
        vsel = W3[:, rows][:, :, qpad, :].reshape(IK, V12_LOCS, B).astype(np.float16)
        w0 = wsel[:128].reshape(128, V12_LOCS * CO)
        v0 = vsel[:128].reshape(128, V12_LOCS * B)
        w1 = wsel[128:256].reshape(128, V12_LOCS * CO)
        v1 = vsel[128:256].reshape(128, V12_LOCS * B)
        wx = np.concatenate([w0, v0, w1, v1], axis=1)
        # c2 [32, (bl, 128 w-cols | 32 v-cols)]
        w2 = wsel[256:].reshape(32, V12_NBLK, V12_BLK * CO)
        v2 = vsel[256:].reshape(32, V12_NBLK, V12_BLK * B)
        c2 = np.concatenate([w2, v2], axis=2).reshape(32, V12_C2COLS)
        in_maps.append({"wx": np.ascontiguousarray(wx),
                        "c2": np.ascontiguousarray(c2)})
    return in_maps


def _assemble_v12(results):
    out = np.empty((B, CO, HO, WO), np.float32)
    idx = np.arange(V12_BLK)
    for c in range(N_CORES):
        nreal = RPC if c < N_CORES - 1 else HO - 4 * (N_CORES - 1)
        buf = np.asarray(results[c]["out"]).astype(np.float32)   # [32, 4096]
        b6 = buf.reshape(V12_BLK, B, RPC, 8, V12_BLK, CO)
        d = b6[idx, :, :, :, idx, :]                 # (BLK, B, RPC, 8, CO)
        dd = d.transpose(1, 4, 2, 3, 0).reshape(B, CO, RPC, 32)
        out[:, :, 4 * c:4 * c + nreal, :] = dd[:, :, :nreal, :WO]
    return out


# ---------------------------------------------------------------------------
# v13: single-iteration critical-path design. For_i iterations are barrier-
# separated (no cross-iteration overlap), so per-iter time = first-DMA
# descriptor time (~18ns/partition-row) + total HBM stream (~330 GB/s
# aggregate, ring-shared) + last-piece compute tail + copy/out tail.
# - BLK=8 locs per block, 16 blocks; quarter-major input layout so compute
#   on quarter q overlaps the stream of quarter q+1.
# - Contraction 288 = 128 + 128 + 32; the 32-row c2 lives in its own
#   [32, 5120] tensor on the scalar ring (descriptor gen runs per-ring).
#   c2 matmuls (K=32, slow to pipeline) are batched per quarter.
# - fp16 output; copies alternate vector/scalar engines.
V13_NBLK = 16
V13_BLK = 8
V13_QCOLS = 2560            # [W0q 1024 | V0q 256 | W1q 1024 | V1q 256]
V13_COLS = 4 * V13_QCOLS
V13_C2COLS = V13_NBLK * 320  # per block: 256 weight | 64 window cols
V13_OROWS = V13_BLK * B     # 64
V13_BOUT = V13_BLK * CO     # 256
V13_OUTC = V13_NBLK * V13_BOUT  # 4096


def _build_nc_v13(repeat=1, cfg="A", out_ring="gpsimd"):
    """cfg A: wx in 1 sync DMA; B: 2 sync halves; C: quarters alternating
    sync/scalar rings. c2 always first on scalar."""
    dt = mybir.dt.float16
    nc = bacc.Bacc("TRN2", target_bir_lowering=False)
    wx = nc.dram_tensor("wx", [128, V13_COLS], dt, kind="ExternalInput")
    c2 = nc.dram_tensor("c2", [32, V13_C2COLS], dt, kind="ExternalInput")
    out = nc.dram_tensor("out", [V13_OROWS, V13_OUTC], dt, kind="ExternalOutput")
    oeng = {"gpsimd": nc.gpsimd, "sync": nc.sync, "scalar": nc.scalar}[out_ring]
    with tile.TileContext(nc) as tc:
        with (
            tc.tile_pool(name="wp", bufs=2) as wp,
            tc.tile_pool(name="cp", bufs=2) as cp,
            tc.tile_pool(name="pp", bufs=2, space="PSUM") as pp,
            tc.tile_pool(name="op", bufs=2) as op,
        ):
            def body():
                t = wp.tile([128, V13_COLS], dt, tag="t", name="t")
                t2 = cp.tile([32, V13_C2COLS], dt, tag="t2", name="t2")
                nc.scalar.dma_start(t2[:], c2.ap()[:, :])
                if cfg == "A":
                    nc.sync.dma_start(t[:], wx.ap()[:, :])
                elif cfg == "B":
                    h = V13_COLS // 2
                    nc.sync.dma_start(t[:, :h], wx.ap()[:, :h])
                    nc.sync.dma_start(t[:, h:], wx.ap()[:, h:])
                else:
                    qc = V13_QCOLS
                    for q in range(4):
                        eng = nc.sync if q % 2 == 0 else nc.scalar
                        eng.dma_start(t[:, q * qc:(q + 1) * qc],
                                      wx.ap()[:, q * qc:(q + 1) * qc])
                ot = op.tile([V13_OROWS, V13_OUTC], dt, tag="ot", name="ot")
                for q in range(4):
                    base = q * V13_QCOLS
                    pss = []
                    for i in range(4):
                        bl = 4 * q + i
                        ps = pp.tile([V13_OROWS, V13_BOUT], mybir.dt.float32,
                                     tag=f"ps{i}", name=f"ps{i}")
                        pss.append(ps)
                        nc.tensor.matmul(
                            ps[:],
                            lhsT=t[:, base + 1024 + 64 * i:base + 1024 + 64 * i + 64],
                            rhs=t[:, base + 256 * i:base + 256 * i + 256],
                            start=True, stop=False)
                        nc.tensor.matmul(
                            ps[:],
                            lhsT=t[:, base + 2304 + 64 * i:base + 2304 + 64 * i + 64],
                            rhs=t[:, base + 1280 + 256 * i:base + 1280 + 256 * i + 256],
                            start=False, stop=False)
                    for i in range(4):
                        bl = 4 * q + i
                        nc.tensor.matmul(
                            pss[i][:],
                            lhsT=t2[:, 320 * bl + 256:320 * bl + 320],
                            rhs=t2[:, 320 * bl:320 * bl + 256],
                            start=False, stop=True)
                    for i in range(4):
                        bl = 4 * q + i
                        dst = ot[:, V13_BOUT * bl:V13_BOUT * (bl + 1)]
                        (nc.vector.tensor_copy if i % 2 == 0 else nc.scalar.copy)(
                            dst, pss[i][:])
                    if q == 2:
                        nc.gpsimd.dma_start(out.ap()[:, :2048], ot[:, :2048])
                oeng.dma_start(out.ap()[:, 2048:], ot[:, 2048:])
            if repeat == 1:
                body()
            else:
                with tc.For_i(0, repeat, 1):
                    body()
    nc.compile()
    return nc


def _host_prep_v13(x, weight):
    x = np.ascontiguousarray(np.asarray(x, dtype=np.float32))
    weight = np.ascontiguousarray(np.asarray(weight, dtype=np.float32))
    wins = np.stack(
        [x[:, :, kh:kh + DH * HO:DH, kw:kw + DW * WO:DW]
         for kh in range(KH) for kw in range(KW)],
        axis=-1,
    )
    W2 = weight[0].transpose(1, 4, 2, 3, 0).reshape(IK, HO, WO, CO)
    W3 = wins.transpose(1, 4, 2, 3, 0).reshape(IK, HO, WO, B)
    qpad = list(range(WO)) + [WO - 1]
    in_maps = []
    for c in range(N_CORES):
        rows = _ROWS_PADDED[c]
        wsel = W2[:, rows][:, :, qpad, :].reshape(IK, 128, CO).astype(np.float16)
        vsel = W3[:, rows][:, :, qpad, :].reshape(IK, 128, B).astype(np.float16)
        qparts = []
        for q in range(4):
            sl = slice(32 * q, 32 * q + 32)
            qparts += [
                wsel[:128, sl].reshape(128, 1024),
                vsel[:128, sl].reshape(128, 256),
                wsel[128:256, sl].reshape(128, 1024),
                vsel[128:256, sl].reshape(128, 256),
            ]
        wx = np.concatenate(qparts, axis=1)
        w2 = wsel[256:].reshape(32, V13_NBLK, V13_BLK * CO)
        v2 = vsel[256:].reshape(32, V13_NBLK, V13_BLK * B)
        c2 = np.concatenate([w2, v2], axis=2).reshape(32, V13_C2COLS)
        in_maps.append({"wx": np.ascontiguousarray(wx),
                        "c2": np.ascontiguousarray(c2)})
    return in_maps


def _assemble_v13(results):
    out = np.empty((B, CO, HO, WO), np.float32)
    idx = np.arange(V13_BLK)
    for c in range(N_CORES):
        nreal = RPC if c < N_CORES - 1 else HO - 4 * (N_CORES - 1)
        buf = np.asarray(results[c]["out"]).astype(np.float32)   # [64, 4096]
        b5 = buf.reshape(V13_BLK, B, V13_NBLK, V13_BLK, CO)
        d = b5[idx, :, :, idx, :]                  # (j, b, bl, o)
        dd = d.transpose(1, 3, 2, 0).reshape(B, CO, RPC, 4 * V13_BLK)
        out[:, :, 4 * c:4 * c + nreal, :] = dd[:, :, :nreal, :WO]
    return out


# ---------------------------------------------------------------------------
# v14: v13 layout + DVE-only copies (ACT copies pull a ~1.4us activation-
# table load into every iteration) + optional paired-c2 scheme:
# cfg "P": per block-PAIR psum [128, 512]; even block -> quadrant
# [0:64, 0:256], odd block -> [64:128, 256:512] (psum out base 64 is legal),
# and ONE c2 matmul per pair (K=32, N=512) accumulating over the whole
# [128, 512] tile — the off-quadrants collect garbage that is never read.
# This halves the count of slow K=32 matmuls and makes the output tile
# [128, 2048] (full-width DMA). cfg "S": v13-style per-block c2.
V14_OROWS_P = 128
V14_OUTC_P = 2048


def _build_nc_v14f(repeat=1, qout=True):
    """v14P but each pair's full [128, 512] psum is drained in ONE DVE copy
    (8 instead of 16 DVE instrs; out doubles to [128, 4096] fp16, shipped
    quarter-wise on gpsimd so only the last 262KB piece is tail)."""
    dt = mybir.dt.float16
    nc = bacc.Bacc("TRN2", target_bir_lowering=False)
    wx = nc.dram_tensor("wx", [128, V13_COLS], dt, kind="ExternalInput")
    c2 = nc.dram_tensor("c2", [32, V13_C2COLS], dt, kind="ExternalInput")
    out = nc.dram_tensor("out", [128, 4096], dt, kind="ExternalOutput")
    with tile.TileContext(nc) as tc:
        with (
            tc.tile_pool(name="wp", bufs=2) as wp,
            tc.tile_pool(name="cp", bufs=2) as cp,
            tc.tile_pool(name="pp", bufs=1, space="PSUM") as pp,
            tc.tile_pool(name="op", bufs=2) as op,
        ):
            def body():
                t = wp.tile([128, V13_COLS], dt, tag="t", name="t")
                t2 = cp.tile([32, V13_C2COLS], dt, tag="t2", name="t2")
                nc.scalar.dma_start(t2[:], c2.ap()[:, :])
                qc = V13_QCOLS
                for q in range(4):
                    eng = nc.sync if q % 2 == 0 else nc.scalar
                    eng.dma_start(t[:, q * qc:(q + 1) * qc],
                                  wx.ap()[:, q * qc:(q + 1) * qc])
                ot = op.tile([128, 4096], dt, tag="ot", name="ot")
                for q in range(4):
                    base = q * qc
                    for pi in (2 * q, 2 * q + 1):
                        ps = pp.tile([128, 512], mybir.dt.float32,
                                     tag=f"pr{pi % 4}", name=f"pr{pi % 4}", bufs=2)
                        for half in range(2):
                            i = (pi % 2) * 2 + half
                            po, co_ = (0, 0) if half == 0 else (64, 256)
                            dst = ps[po:po + 64, co_:co_ + 256]
                            nc.tensor.matmul(
                                dst,
                                lhsT=t[:, base + 1024 + 64 * i:base + 1024 + 64 * i + 64],
                                rhs=t[:, base + 256 * i:base + 256 * i + 256],
                                start=True, stop=False)
                            nc.tensor.matmul(
                                dst,
                                lhsT=t[:, base + 2304 + 64 * i:base + 2304 + 64 * i + 64],
                                rhs=t[:, base + 1280 + 256 * i:base + 1280 + 256 * i + 256],
                                start=False, stop=False)
                        nc.tensor.matmul(
                            ps[:],
                            lhsT=t2[:, 640 * pi + 512:640 * pi + 640],
                            rhs=t2[:, 640 * pi:640 * pi + 512],
                            start=False, stop=True, skip_group_check=True)
                        nc.vector.tensor_copy(
                            ot[:, 512 * pi:512 * (pi + 1)], ps[:])
                    if qout and q < 3:
                        nc.gpsimd.dma_start(
                            out.ap()[:, 1024 * q:1024 * (q + 1)],
                            ot[:, 1024 * q:1024 * (q + 1)])
                if qout:
                    nc.gpsimd.dma_start(out.ap()[:, 3072:], ot[:, 3072:])
                else:
                    nc.gpsimd.dma_start(out.ap()[:, :], ot[:, :])
            if repeat == 1:
                body()
            else:
                with tc.For_i(0, repeat, 1):
                    body()
    nc.compile()
    return nc


def _assemble_v14f(results):
    out = np.empty((B, CO, HO, WO), np.float32)
    for c in range(N_CORES):
        nreal = RPC if c < N_CORES - 1 else HO - 4 * (N_CORES - 1)
        buf = np.asarray(results[c]["out"]).astype(np.float32)   # [128, 4096]
        b5 = buf.reshape(16, B, 8, 16, CO)           # (j, b, pair, j', o)
        d = np.stack([b5[j, :, :, j, :] for j in range(16)])      # (j, b, p, o)
        dd = d.transpose(1, 3, 2, 0).reshape(B, CO, 8, 16)        # l = 16p + j
        dd = dd.reshape(B, CO, RPC, 32)
        out[:, :, 4 * c:4 * c + nreal, :] = dd[:, :, :nreal, :WO]
    return out


# v15: pair-major input layout (1280 cols/pair: [w0 512 | v0 128 | w1 512 |
# v1 128]) cut into UNEVEN DMA pieces 3+2+2+1 pairs, so the last piece is a
# single pair: the post-last-DMA tail (compute+copies of what the final
# piece gates) halves vs v14P's 2-pair quarter.
V15_PIECES = ((0, 3), (3, 2), (5, 2), (7, 1))   # (first pair, n pairs)


def _build_nc_v15(repeat=1, pieces=V15_PIECES):
    dt = mybir.dt.float16
    nc = bacc.Bacc("TRN2", target_bir_lowering=False)
    wx = nc.dram_tensor("wx", [128, V13_COLS], dt, kind="ExternalInput")
    c2 = nc.dram_tensor("c2", [32, V13_C2COLS], dt, kind="ExternalInput")
    out = nc.dram_tensor("out", [V14_OROWS_P, V14_OUTC_P], dt, kind="ExternalOutput")
    with tile.TileContext(nc) as tc:
        with (
            tc.tile_pool(name="wp", bufs=2) as wp,
            tc.tile_pool(name="cp", bufs=2) as cp,
            tc.tile_pool(name="pp", bufs=1, space="PSUM") as pp,
            tc.tile_pool(name="op", bufs=2) as op,
        ):
            def body():
                t = wp.tile([128, V13_COLS], dt, tag="t", name="t")
                t2 = cp.tile([32, V13_C2COLS], dt, tag="t2", name="t2")
                nc.scalar.dma_start(t2[:], c2.ap()[:, :])
                for d_i, (p0, np_) in enumerate(pieces):
                    eng = nc.sync if d_i % 2 == 0 else nc.scalar
                    eng.dma_start(t[:, 1280 * p0:1280 * (p0 + np_)],
                                  wx.ap()[:, 1280 * p0:1280 * (p0 + np_)])
                ot = op.tile([V14_OROWS_P, V14_OUTC_P], dt, tag="ot", name="ot")
                for pi in range(8):
                    base = 1280 * pi
                    ps = pp.tile([128, 512], mybir.dt.float32,
                                 tag=f"pr{pi % 4}", name=f"pr{pi % 4}", bufs=2)
                    for h in range(2):
                        po, co_ = (0, 0) if h == 0 else (64, 256)
                        dst = ps[po:po + 64, co_:co_ + 256]
                        nc.tensor.matmul(
                            dst,
                            lhsT=t[:, base + 512 + 64 * h:base + 512 + 64 * h + 64],
                            rhs=t[:, base + 256 * h:base + 256 * h + 256],
                            start=True, stop=False)
                        nc.tensor.matmul(
                            dst,
                            lhsT=t[:, base + 1152 + 64 * h:base + 1152 + 64 * h + 64],
                            rhs=t[:, base + 640 + 256 * h:base + 640 + 256 * h + 256],
                            start=False, stop=False)
                    nc.tensor.matmul(
                        ps[:],
                        lhsT=t2[:, 640 * pi + 512:640 * pi + 640],
                        rhs=t2[:, 640 * pi:640 * pi + 512],
                        start=False, stop=True, skip_group_check=True)
                    nc.vector.tensor_copy(
                        ot[0:64, 256 * pi:256 * pi + 256], ps[0:64, 0:256])
                    nc.vector.tensor_copy(
                        ot[64:128, 256 * pi:256 * pi + 256], ps[64:128, 256:512])
                    if pi == 5:
                        nc.gpsimd.dma_start(out.ap()[:, :1536], ot[:, :1536])
                nc.gpsimd.dma_start(out.ap()[:, 1536:], ot[:, 1536:])
            if repeat == 1:
                body()
            else:
                with tc.For_i(0, repeat, 1):
                    body()
    nc.compile()
    return nc


def _host_prep_v15(x, weight):
    maps = _host_prep_v14(x, weight, "P")
    out_maps = []
    for m in maps:
        wx = m["wx"]                      # quarter-major [128, 10240]
        qs = wx.reshape(128, 4, 4, 640)   # (part, quarter, region?, ...)
        # quarter layout: [W0q 1024 | V0q 256 | W1q 1024 | V1q 256]
        q4 = wx.reshape(128, 4, 2560)
        pieces = []
        for pi in range(8):
            q, half = pi // 2, pi % 2
            w0 = q4[:, q, 512 * half:512 * half + 512]
            v0 = q4[:, q, 1024 + 128 * half:1024 + 128 * half + 128]
            w1 = q4[:, q, 1280 + 512 * half:1280 + 512 * half + 512]
            v1 = q4[:, q, 2304 + 128 * half:2304 + 128 * half + 128]
            pieces.append(np.concatenate([w0, v0, w1, v1], axis=1))
        wx2 = np.ascontiguousarray(np.concatenate(pieces, axis=1))
        out_maps.append({"wx": wx2, "c2": m["c2"]})
    return out_maps


def _build_nc_v14(repeat=1, cfg="P", skip_c2=False, in_halves=False, qout=False,
                  c2_gpsimd=False, act_copies=False):
    dt = mybir.dt.float16
    nc = bacc.Bacc("TRN2", target_bir_lowering=False)
    wx = nc.dram_tensor("wx", [128, V13_COLS], dt, kind="ExternalInput")
    c2 = nc.dram_tensor("c2", [32, V13_C2COLS], dt, kind="ExternalInput")
    if cfg == "P":
        out = nc.dram_tensor("out", [V14_OROWS_P, V14_OUTC_P], dt, kind="ExternalOutput")
    else:
        out = nc.dram_tensor("out", [V13_OROWS, V13_OUTC], dt, kind="ExternalOutput")
    with tile.TileContext(nc) as tc:
        with (
            tc.tile_pool(name="wp", bufs=2) as wp,
            tc.tile_pool(name="cp", bufs=2) as cp,
            tc.tile_pool(name="pp", bufs=1, space="PSUM") as pp,
            tc.tile_pool(name="op", bufs=2) as op,
        ):
            def body():
                t = wp.tile([128, V13_COLS], dt, tag="t", name="t")
                t2 = cp.tile([32, V13_C2COLS], dt, tag="t2", name="t2")
                (nc.gpsimd if c2_gpsimd else nc.scalar).dma_start(t2[:], c2.ap()[:, :])
                qc = V13_QCOLS
                if in_halves:
                    nc.sync.dma_start(t[:, :2 * qc], wx.ap()[:, :2 * qc])
                    nc.scalar.dma_start(t[:, 2 * qc:], wx.ap()[:, 2 * qc:])
                else:
                    for q in range(4):
                        eng = nc.sync if q % 2 == 0 else nc.scalar
                        eng.dma_start(t[:, q * qc:(q + 1) * qc],
                                      wx.ap()[:, q * qc:(q + 1) * qc])
                if cfg == "P":
                    ot = op.tile([V14_OROWS_P, V14_OUTC_P], dt, tag="ot", name="ot")
                    for q in range(4):
                        base = q * qc
                        for pi in (2 * q, 2 * q + 1):   # pairs in this quarter
                            ps = pp.tile([128, 512], mybir.dt.float32,
                                         tag=f"pr{pi % 4}", name=f"pr{pi % 4}", bufs=2)
                            for half in range(2):       # even/odd block of pair
                                i = (pi % 2) * 2 + half  # block index in quarter
                                po, co_ = (0, 0) if half == 0 else (64, 256)
                                dst = ps[po:po + 64, co_:co_ + 256]
                                nc.tensor.matmul(
                                    dst,
                                    lhsT=t[:, base + 1024 + 64 * i:base + 1024 + 64 * i + 64],
                                    rhs=t[:, base + 256 * i:base + 256 * i + 256],
                                    start=True, stop=False)
                                nc.tensor.matmul(
                                    dst,
                                    lhsT=t[:, base + 2304 + 64 * i:base + 2304 + 64 * i + 64],
                                    rhs=t[:, base + 1280 + 256 * i:base + 1280 + 256 * i + 256],
                                    start=False, stop=False)
                            if skip_c2:
                                # timing ablation: same-shape matmul but K=128
                                nc.tensor.matmul(
                                    ps[:],
                                    lhsT=t[:, base:base + 128],
                                    rhs=t[:, base + 512:base + 1024],
                                    start=False, stop=True, skip_group_check=True)
                            else:
                                nc.tensor.matmul(
                                    ps[:],
                                    lhsT=t2[:, 640 * pi + 512:640 * pi + 640],
                                    rhs=t2[:, 640 * pi:640 * pi + 512],
                                    start=False, stop=True, skip_group_check=True)
                            cp_a = nc.scalar.copy if (act_copies and pi % 2 == 1) \
                                else nc.vector.tensor_copy
                            cp_a(ot[0:64, 256 * pi:256 * pi + 256], ps[0:64, 0:256])
                            cp_a(ot[64:128, 256 * pi:256 * pi + 256], ps[64:128, 256:512])
                        if qout and q < 3:
                            nc.gpsimd.dma_start(
                                out.ap()[:, 512 * q:512 * (q + 1)],
                                ot[:, 512 * q:512 * (q + 1)])
                        elif not qout and q == 2:
                            nc.gpsimd.dma_start(out.ap()[:, :1024], ot[:, :1024])
                    if qout:
                        nc.gpsimd.dma_start(out.ap()[:, 1536:], ot[:, 1536:])
                    else:
                        nc.gpsimd.dma_start(out.ap()[:, 1024:], ot[:, 1024:])
                else:
                    ot = op.tile([V13_OROWS, V13_OUTC], dt, tag="ot", name="ot")
                    for q in range(4):
                        base = q * qc
                        pss = []
                        for i in range(4):
                            ps = pp.tile([V13_OROWS, V13_BOUT], mybir.dt.float32,
                                         tag=f"ps{i}", name=f"ps{i}", bufs=2)
                            pss.append(ps)
                            nc.tensor.matmul(
                                ps[:],
                                lhsT=t[:, base + 1024 + 64 * i:base + 1024 + 64 * i + 64],
                                rhs=t[:, base + 256 * i:base + 256 * i + 256],
                                start=True, stop=False)
                            nc.tensor.matmul(
                                ps[:],
                                lhsT=t[:, base + 2304 + 64 * i:base + 2304 + 64 * i + 64],
                                rhs=t[:, base + 1280 + 256 * i:base + 1280 + 256 * i + 256],
                                start=False, stop=False)
                        for i in range(4):
                            bl = 4 * q + i
                            nc.tensor.matmul(
                                pss[i][:],
                                lhsT=t2[:, 320 * bl + 256:320 * bl + 320],
                                rhs=t2[:, 320 * bl:320 * bl + 256],
                                start=False, stop=True)
                        for i in range(4):
                            bl = 4 * q + i
                            nc.vector.tensor_copy(
                                ot[:, V13_BOUT * bl:V13_BOUT * (bl + 1)], pss[i][:])
                        if q == 2:
                            nc.gpsimd.dma_start(out.ap()[:, :2048], ot[:, :2048])
                    nc.gpsimd.dma_start(out.ap()[:, 2048:], ot[:, 2048:])
            if repeat == 1:
                body()
            else:
                with tc.For_i(0, repeat, 1):
                    body()
    nc.compile()
    return nc


def _host_prep_v14(x, weight, cfg="P"):
    in_maps = _host_prep_v13(x, weight)
    if cfg not in ("P", "F"):
        return in_maps
    out_maps = []
    for m in in_maps:
        c2 = m["c2"].reshape(32, V13_NBLK, 320)
        w = c2[:, :, :256].reshape(32, 8, 2 * 256)      # (r, pair, 512 w-cols)
        v = c2[:, :, 256:].reshape(32, 8, 2 * 64)       # (r, pair, 128 v-cols)
        c2p = np.concatenate([w, v], axis=2).reshape(32, V13_C2COLS)
        out_maps.append({"wx": m["wx"], "c2": np.ascontiguousarray(c2p)})
    return out_maps


def _assemble_v14(results, cfg="P"):
    if cfg != "P":
        return _assemble_v13(results)
    out = np.empty((B, CO, HO, WO), np.float32)
    for c in range(N_CORES):
        nreal = RPC if c < N_CORES - 1 else HO - 4 * (N_CORES - 1)
        buf = np.asarray(results[c]["out"]).astype(np.float32)   # [128, 2048]
        b5 = buf.reshape(16, B, 8, 8, CO)            # (j, b, pair, j', o)
        d = np.stack([b5[j, :, :, j % 8, :] for j in range(16)])  # (j, b, p, o)
        dd = d.transpose(1, 3, 2, 0).reshape(B, CO, 8, 16)        # l = 16p + j
        dd = dd.reshape(B, CO, RPC, 32)
        out[:, :, 4 * c:4 * c + nreal, :] = dd[:, :, :nreal, :WO]
    return out


V10_GTOT = NCHUNK * GLP * CO + NCHUNK * GLP * B   # 3840 cols/group: weight | windows


def _build_nc_v10(repeat=1, dt=None):
    """Like v8h but weight+windows interleaved per group in ONE DRAM tensor:
    one DMA per group (4 input DMAs total) — each dma_start costs ~1.5us of
    serialized ring time here, so DMA count is the dominant knob."""
    if dt is None:
        dt = mybir.dt.float16
    BLK = V89_BLK
    NBLK = V89_NBLK
    gw = V89_GW
    gtot = V10_GTOT
    bout = V89_BOUT
    orows = V89_OROWS
    nc = bacc.Bacc("TRN2", target_bir_lowering=False)
    wx = nc.dram_tensor("wx", [CK, GROUPS * gtot], dt, kind="ExternalInput")
    out = nc.dram_tensor("out", [orows, V89_OUTC], mybir.dt.float32, kind="ExternalOutput")
    with tile.TileContext(nc) as tc:
        with (
            tc.tile_pool(name="wp", bufs=2) as wp,
            tc.tile_pool(name="pp", bufs=4, space="PSUM") as pp,
            tc.tile_pool(name="op", bufs=2) as op,
        ):
            def body():
                t = wp.tile([CK, GROUPS * gtot], dt, tag="t", name="t")
                for g in range(GROUPS):
                    nc.sync.dma_start(t[:, g * gtot:(g + 1) * gtot],
                                      wx.ap()[:, g * gtot:(g + 1) * gtot])
                ot = op.tile([orows, V89_OUTC], mybir.dt.float32, tag="ot", name="ot")
                gout = NBLK * bout
                for g in range(GROUPS):
                    base = g * gtot
                    for bl in range(NBLK):
                        ps = pp.tile([orows, bout], mybir.dt.float32, tag="ps", name="ps")
                        for c in range(NCHUNK):
                            lo = base + gw + c * (GLP * B) + bl * (BLK * B)
                            ro = base + c * (GLP * CO) + bl * bout
                            nc.tensor.matmul(
                                ps[:],
                                lhsT=t[:, lo:lo + BLK * B],
                                rhs=t[:, ro:ro + bout],
                                start=(c == 0), stop=(c == NCHUNK - 1))
                        nc.vector.tensor_copy(
                            ot[:, (g * NBLK + bl) * bout:(g * NBLK + bl + 1) * bout], ps[:])
                    if g == GROUPS - 2:
                        # first 3/4 of the output leaves while group 3 computes
                        nc.gpsimd.dma_start(out.ap()[:, :3 * gout], ot[:, :3 * gout])
                nc.gpsimd.dma_start(out.ap()[:, 3 * gout:], ot[:, 3 * gout:])
            if repeat == 1:
                body()
            else:
                with tc.For_i(0, repeat, 1):
                    body()
    nc.compile()
    return nc


def _host_prep_v10(x, weight, npdt=None):
    if npdt is None:
        npdt = np.float16
    maps = _host_prep_v89(x, weight, three_term=False, npdt=npdt)
    gw = V89_GW
    gwin = V89_GWIN
    out_maps = []
    for m in maps:
        wh = m["wh"].reshape(CK, GROUPS, gw)
        vh = m["winh"].reshape(CK, GROUPS, gwin)
        wx = np.concatenate([wh, vh], axis=2).reshape(CK, GROUPS * V10_GTOT)
        out_maps.append({"wx": np.ascontiguousarray(wx)})
    return out_maps


def _host_prep_v4(x, weight, variant):
    dt, BLK, npdt = V4_CFG[variant]
    x = np.ascontiguousarray(np.asarray(x, dtype=np.float32))
    weight = np.ascontiguousarray(np.asarray(weight, dtype=np.float32))
    wins = np.stack(
        [x[:, :, kh:kh + DH * HO:DH, kw:kw + DW * WO:DW]
         for kh in range(KH) for kw in range(KW)],
        axis=-1,
    )
    W2 = weight[0].transpose(1, 4, 2, 3, 0).reshape(IK, HO, WO, CO)
    W3 = wins.transpose(1, 4, 2, 3, 0).reshape(IK, HO, WO, B)
    qpad = list(range(WO)) + [WO - 1]          # 31 real + 1 dup -> 32
    in_maps = []
    for c in range(N_CORES):
        rows = _ROWS_PADDED[c]
        # (ik, group, locp, {o|b})
        wsel = W2[:, rows][:, :, qpad, :]       # (288, 4, 32, CO)
        winsel = W3[:, rows][:, :, qpad, :]     # (288, 4, 32, B)
        # -> [group, CK, chunk, locp, {o|b}] -> [GROUPS*CK, chunk*locp*{o|b}]
        wstk = np.stack([wsel[CK * cc:CK * (cc + 1)] for cc in range(NCHUNK)], axis=2)
        winstk = np.stack([winsel[CK * cc:CK * (cc + 1)] for cc in range(NCHUNK)], axis=2)
        # wstk: (CK, 4, chunk, 32, CO) -> (4, CK, chunk, 32, CO)
        wstk = wstk.transpose(1, 0, 2, 3, 4).reshape(GROUPS * CK, NCHUNK * GLP * CO)
        winstk = winstk.transpose(1, 0, 2, 3, 4).reshape(GROUPS * CK, NCHUNK * GLP * B)
        in_maps.append({
            "wT": np.ascontiguousarray(wstk.astype(npdt)),
            "winT": np.ascontiguousarray(winstk.astype(npdt)),
        })
    return in_maps


def _assemble_v4(results, variant):
    dt, BLK, _ = V4_CFG[variant]
    NBLK = GLP // BLK
    out = np.empty((B, CO, HO, WO), np.float32)
    idx = np.arange(BLK)
    for c in range(N_CORES):
        nreal = RPC if c < N_CORES - 1 else HO - 4 * (N_CORES - 1)
        buf = np.asarray(results[c]["out"])
        b6 = buf.reshape(BLK, B, GROUPS, NBLK, BLK, CO)
        d = b6[idx, :, :, :, idx, :]            # (BLK, B, GROUPS, NBLK, CO)
        dd = d.transpose(1, 4, 2, 3, 0).reshape(B, CO, GROUPS, NBLK * BLK)
        out[:, :, 4 * c:4 * c + nreal, :] = dd[:, :, :nreal, :WO]
    return out


def _build_nc(repeat=1, variant="v2"):
    nc = bacc.Bacc("TRN2", target_bir_lowering=False)
    wT = nc.dram_tensor("wT", [CK, W_COLS], mybir.dt.float32, kind="ExternalInput")
    winT = nc.dram_tensor("winT", [CK, WIN_COLS], mybir.dt.float32, kind="ExternalInput")
    out_cols = OUT_COLS if variant == "v1" else V2_OUT_COLS
    out_rows = CO if variant == "v1" else 128
    out = nc.dram_tensor("out", [out_rows, out_cols], mybir.dt.float32, kind="ExternalOutput")

    gw = GL * NCHUNK * CO    # weight cols per group
    gwin = GL * NCHUNK * B   # window cols per group
    gout = GL * B            # v1 out cols per group

    with tile.TileContext(nc) as tc:
        with (
            tc.tile_pool(name="wp", bufs=3) as wp,
            tc.tile_pool(name="winp", bufs=3) as winp,
            tc.tile_pool(name="pp", bufs=2, space="PSUM") as pp,
            tc.tile_pool(name="op", bufs=2) as op,
        ):
            def body_v1():
                for g in range(GROUPS):
                    wt = wp.tile([CK, gw], mybir.dt.float32, tag="wt", name="wt")
                    nc.sync.dma_start(wt[:], wT.ap()[:, g * gw:(g + 1) * gw])
                    wint = winp.tile([CK, gwin], mybir.dt.float32, tag="wint", name="wint")
                    nc.sync.dma_start(wint[:], winT.ap()[:, g * gwin:(g + 1) * gwin])

                    ps = pp.tile([CO, gout], mybir.dt.float32, tag="ps", name="ps")
                    for l in range(GL):
                        for c in range(NCHUNK):
                            nc.tensor.matmul(
                                ps[:, l * B:(l + 1) * B],
                                lhsT=wt[:, (l * NCHUNK + c) * CO:(l * NCHUNK + c + 1) * CO],
                                rhs=wint[:, (l * NCHUNK + c) * B:(l * NCHUNK + c + 1) * B],
                                start=(c == 0),
                                stop=(c == NCHUNK - 1),
                            )

                    ot = op.tile([CO, gout], mybir.dt.float32, tag="ot", name="ot")
                    nc.vector.tensor_copy(ot[:], ps[:])
                    nc.sync.dma_start(out.ap()[:, g * gout:(g + 1) * gout], ot[:])

            def body_v2():
                # stationary = windows (8 cols, cheap fp32 self-load);
                # moving = weight (N=32); out[b, o] block at partition
                # offset 32*(l%4) via col-tiling -> 4 concurrent MM strips.
                for g in range(GROUPS):
                    wt = wp.tile([CK, gw], mybir.dt.float32, tag="wt", name="wt")
                    nc.sync.dma_start(wt[:], wT.ap()[:, g * gw:(g + 1) * gw])
                    wint = winp.tile([CK, gwin], mybir.dt.float32, tag="wint", name="wint")
                    nc.sync.dma_start(wint[:], winT.ap()[:, g * gwin:(g + 1) * gwin])

                    # one PSUM tile per col strip -> different banks, so the
                    # 4 strips' matmuls aren't serialized by bank tracking
                    pss = [
                        pp.tile([128, V2_GOUT], mybir.dt.float32,
                                tag=f"ps{j}", name=f"ps{j}", bufs=2)
                        for j in range(4)
                    ]
                    for l in range(GL):
                        j = l % 4
                        blk = l // 4
                        for c in range(NCHUNK):
                            nc.tensor.matmul(
                                pss[j][32 * j:32 * j + B, blk * CO:(blk + 1) * CO],
                                lhsT=wint[:, (l * NCHUNK + c) * B:(l * NCHUNK + c + 1) * B],
                                rhs=wt[:, (l * NCHUNK + c) * CO:(l * NCHUNK + c + 1) * CO],
                                start=(c == 0),
                                stop=(c == NCHUNK - 1),
                                tile_position=(0, 32 * j),
                            )

                    ot = op.tile([128, V2_GOUT], mybir.dt.float32, tag="ot", name="ot")
                    for j in range(4):
                        nc.vector.tensor_copy(
                            ot[32 * j:32 * (j + 1), :],
                            pss[j][32 * j:32 * (j + 1), :],
                        )
                    nc.sync.dma_start(out.ap()[:, g * V2_GOUT:(g + 1) * V2_GOUT], ot[:])

            body = body_v1 if variant == "v1" else body_v2
            if repeat == 1:
                body()
            else:
                with tc.For_i(0, repeat, 1):
                    body()
    nc.compile()
    return nc


def _host_prep(x, weight):
    """Build per-core DMA-ready layouts. Pure indexing/transpose, no math."""
    x = np.ascontiguousarray(np.asarray(x, dtype=np.float32))
    weight = np.ascontiguousarray(np.asarray(weight, dtype=np.float32))

    # windows[b, i, p, q, k] with k = kh*3+kw (matches torch unfold flatten)
    wins = np.stack(
        [x[:, :, kh:kh + DH * HO:DH, kw:kw + DW * WO:DW]
         for kh in range(KH) for kw in range(KW)],
        axis=-1,
    )  # (B, CI, HO, WO, 9)

    # (ik, p, q, o) and (ik, p, q, b)
    W2 = weight[0].transpose(1, 4, 2, 3, 0).reshape(IK, HO, WO, CO)
    W3 = wins.transpose(1, 4, 2, 3, 0).reshape(IK, HO, WO, B)

    in_maps = []
    for c in range(N_CORES):
        rows = _ROWS_PADDED[c]
        wsel = W2[:, rows].reshape(IK, L, CO)
        winsel = W3[:, rows].reshape(IK, L, B)
        # [CK, loc, chunk, {o|b}] — partition r of chunk-c col region holds ik=96c+r
        wT = np.stack([wsel[CK * cc:CK * (cc + 1)] for cc in range(NCHUNK)], axis=2)
        winT = np.stack([winsel[CK * cc:CK * (cc + 1)] for cc in range(NCHUNK)], axis=2)
        in_maps.append({
            "wT": np.ascontiguousarray(wT.reshape(CK, W_COLS)),
            "winT": np.ascontiguousarray(winT.reshape(CK, WIN_COLS)),
        })
    return in_maps


def _assemble(results, variant="v2"):
    out = np.empty((B, CO, HO, WO), np.float32)
    qs = np.arange(WO)
    for c in range(N_CORES):
        nreal = RPC if c < N_CORES - 1 else HO - 4 * (N_CORES - 1)
        buf = np.asarray(results[c]["out"])
        if variant == "v1":
            rr = buf.reshape(CO, RPC, WO, B)
            for j in range(nreal):
                out[:, :, 4 * c + j, :] = rr[:, j, :, :].transpose(2, 0, 1)
        else:
            # buf [128, GROUPS*256]: row = 32*(q%4)+b, col = g*256+(q//4)*32+o
            b4 = buf.reshape(4, 32, GROUPS, 8, CO)
            res = b4[qs % 4, :B, :, qs // 4, :]      # (31, b, g, o)
            out[:, :, 4 * c:4 * c + nreal, :] = res.transpose(1, 3, 2, 0)[:, :, :nreal, :]
    return out


VARIANT = os.environ.get("LC2D_VARIANT", "v14P")


def kernel(x, weight, _trace=False, _trace_cores=None):
    if VARIANT == "v15":
        in_maps = _host_prep_v15(x, weight)
    elif VARIANT.startswith("v14"):
        in_maps = _host_prep_v14(x, weight, cfg=VARIANT[3:] or "P")
    elif VARIANT.startswith("v13"):
        in_maps = _host_prep_v13(x, weight)
    elif VARIANT.startswith("v12"):
        in_maps = _host_prep_v12(x, weight)
    elif VARIANT == "v10":
        in_maps = _host_prep_v10(x, weight)
    elif VARIANT in ("v8", "v9", "v8h", "v9h"):
        in_maps = _host_prep_v89(
            x, weight, three_term=(VARIANT in ("v9", "v9h")),
            npdt=(np.float16 if VARIANT.endswith("h") else ml_dtypes.bfloat16))
    elif VARIANT in V4_CFG:
        in_maps = _host_prep_v4(x, weight, VARIANT)
    elif VARIANT == "v5":
        in_maps = _host_prep_v5(x, weight)
    else:
        in_maps = _host_prep(x, weight)
    if "nc" not in _NC_CACHE:
        if VARIANT == "v15":
            _NC_CACHE["nc"] = _build_nc_v15(1)
        elif VARIANT == "v14F":
            _NC_CACHE["nc"] = _build_nc_v14f(1)
        elif VARIANT.startswith("v14"):
            _NC_CACHE["nc"] = _build_nc_v14(1, cfg=VARIANT[3:] or "P")
        elif VARIANT.startswith("v13"):
            _NC_CACHE["nc"] = _build_nc_v13(
                1, cfg=(VARIANT[3:] or "A"))
        elif VARIANT.startswith("v12"):
            _NC_CACHE["nc"] = _build_nc_v12(
                1, n_in_dma=(4 if VARIANT == "v12q" else 1))
        elif VARIANT == "v10":
            _NC_CACHE["nc"] = _build_nc_v10(1)
        elif VARIANT in ("v8", "v9", "v8h", "v9h"):
            _NC_CACHE["nc"] = _build_nc_v89(
                1, three_term=(VARIANT in ("v9", "v9h")),
                dt=(mybir.dt.float16 if VARIANT.endswith("h") else mybir.dt.bfloat16))
        elif VARIANT in V4_CFG:
            _NC_CACHE["nc"] = _build_nc_v4(1, VARIANT)
        elif VARIANT == "v5":
            _NC_CACHE["nc"] = _build_nc_v5()
        else:
            _NC_CACHE["nc"] = _build_nc(variant=VARIANT)
    nc = _NC_CACHE["nc"]
    res = run_bass_kernel_spmd(
        nc, in_maps, core_ids=list(range(N_CORES)),
        trace=_trace, trace_cores=_trace_cores,
    )
    if VARIANT == "v15":
        out = _assemble_v14(res.results, cfg="P")
    elif VARIANT == "v14F":
        out = _assemble_v14f(res.results)
    elif VARIANT.startswith("v14"):
        out = _assemble_v14(res.results, cfg=VARIANT[3:] or "P")
    elif VARIANT.startswith("v13"):
        out = _assemble_v13(res.results)
    elif VARIANT.startswith("v12"):
        out = _assemble_v12(res.results)
    elif VARIANT in ("v8", "v9", "v8h", "v9h", "v10"):
        out = _assemble_v89(res.results)
    elif VARIANT in V4_CFG:
        out = _assemble_v4(res.results, VARIANT)
    elif VARIANT == "v5":
        out = _assemble_v5(res.results)
    else:
        out = _assemble(res.results, variant=VARIANT)
    if _trace:
        return out, res
    return out


if __name__ == "__main__":
    # quick self-check with random data against a numpy oracle
    rng = np.random.default_rng(0)
    x = rng.standard_normal((B, CI, H, W), dtype=np.float32)
    weight = rng.standard_normal((1, CO, CI, HO, WO, KH * KW), dtype=np.float32)
    wins = np.stack(
        [x[:, :, kh:kh + DH * HO:DH, kw:kw + DW * WO:DW]
         for kh in range(KH) for kw in range(KW)], axis=-1)
    expected = np.einsum("bipqk,oipqk->bopq", wins, weight[0], optimize=True)
    actual = kernel(x, weight)
    err = np.abs(actual - expected).max() / np.abs(expected).max()
    print("max out:", np.abs(expected).max(), "rel err:", err)
    tol = 1e-5 if VARIANT in ("v1", "v2", "v5") else (1e-2 if VARIANT in ("v8", "v4b", "v4b8") else 1e-3)
    assert err < tol, (err, tol)
    print("KERNEL OK")

